# revision 14
# baseline (speedup 1.0000x reference)
"""Trainium2 Bass kernel for nn_CAA_Stable (stable-diffusion style channel
self-attention block over 64x64 feature maps).

Reference computation per batch b (C=256 channels, N=64*64=4096 positions):
    q = scale*(Wq@x + bq)  [D=16, N]   (scale folded into q)
    k = Wk@x + bk          [D, N]
    logits[n,m] = q[:,n].k[:,m];  w = softmax(logits, axis=m)
    y = gamma_clipped * (Wo @ ((Wv@x+bv) @ w^T) + bo) + x
Fusions / tricks:
  * Wo @ (V@w^T) == (Wo@V) @ w^T: precompute U = Wo@(Wv@x+bv) (tiny), so the
    big attention-value matmul directly produces the projected output.
  * softmax denominator = ones^T @ exp on the TensorEngine with a [128,128]
    ones stationary: every PSUM partition holds the per-query sum, so one DVE
    reciprocal yields the broadcast normalization tile directly. 10 of 16
    key-groups accumulate their sums on the (otherwise slack) VectorEngine
    instead and fold in via one fp32 ones-matmul, offloading ~34us from the
    bottleneck TensorEngine (split ratio tuned to balance PE and DVE).
  * gamma and bo fold into the epilogue ACT (per-partition scale/bias).
  * av_f8=1 (experimental, off by default): attention-value matmul in
    fp8e4m3 DoubleRow with U split hi+lo and exp(logit-4). Numerically
    verified (rel err 2.3e-4) but most fp8 exp values land subnormal, which
    triggered pathological runtime slowdowns in this environment, so the
    default path keeps the attention matmuls in bf16.
  * QK^T contraction (K=16, zero-padded to 32) uses tile_position row
    packing; consecutive 2-chunk groups alternate row-group pairs {0,1}/{2,3}
    so their matmuls can overlap in the PE array.
PSUM (8 banks): qk 2x[128,1024] double-buffered (4) + ua 2x[128,512] (2) +
den [128,512] (1) + phase-0 scratch (1; phase 0 alternates den/scratch).

Sharding: pure data-parallel over batch, one image per NeuronCore, no
collectives. kernel() takes FULL inputs, returns the FULL output.
"""

import numpy as np

B, C, HW, D = 8, 256, 4096, 16
P = 128
QS = 512              # q-strip width (one PSUM bank)
NSTRIP = HW // QS     # 8
NKC = HW // P         # 32 key chunks of 128
KGRP = 2              # key chunks per group (one double-buffered qk tile)
NGRP = NKC // KGRP    # 16
SCALE = float(D) ** -0.5

_cache = {}


def _build(den_pe_groups=6, repeat=1, loop_repeat=0, av_f8=0, qk_pack=1, dbg=0):
    import ml_dtypes
    import concourse.bacc as bacc
    import concourse.mybir as mybir
    import concourse.tile as tile

    dt = mybir.dt
    AF = mybir.ActivationFunctionType
    ALU = mybir.AluOpType
    f32, bf16, f8 = dt.float32, dt.bfloat16, dt.float8e4
    f8e5 = dt.float8e5
    f32r = dt.float32r
    # av_f8: 0 = bf16 attention-value matmul; 1 = e4m3 hi+lo split (legacy,
    # subnormal-heavy); 2 = e5m2 single-term (range covers exp(logit) for
    # this data with no offset, no subnormals).
    exdt = {0: bf16, 1: f8, 2: f8e5}[av_f8]
    # row-group packing of the QK matmuls (tile_position) — independent of
    # the AV perf mode; qk_pack=0 forces the unpacked row-group-0 layout.
    qkpack = bool(qk_pack) if av_f8 else True
    PM = mybir.MatmulPerfMode

    nc = bacc.Bacc("TRN2", target_bir_lowering=False, debug=False, num_devices=B)

    x_d = nc.dram_tensor("x", [C, HW], f32, kind="ExternalInput")
    wq_d = nc.dram_tensor("Wq", [D, C], f32, kind="ExternalInput")
    bq_d = nc.dram_tensor("bq", [D], f32, kind="ExternalInput")
    wk_d = nc.dram_tensor("Wk", [D, C], f32, kind="ExternalInput")
    bk_d = nc.dram_tensor("bk", [D], f32, kind="ExternalInput")
    wv_d = nc.dram_tensor("Wv", [C, C], f32, kind="ExternalInput")
    bv_d = nc.dram_tensor("bv", [C], f32, kind="ExternalInput")
    wo_d = nc.dram_tensor("Wo", [C, C], f32, kind="ExternalInput")
    bo_d = nc.dram_tensor("bo", [C], f32, kind="ExternalInput")
    g_d = nc.dram_tensor("gamma", [1], f32, kind="ExternalInput")
    y_d = nc.dram_tensor("y", [C, HW], f32, kind="ExternalOutput")
    if dbg:
        dqk_d = nc.dram_tensor("dqk", [P, KGRP * QS], f32, kind="ExternalOutput")
        dex_d = nc.dram_tensor("dex", [P, KGRP * QS], f32, kind="ExternalOutput")
        dden_d = nc.dram_tensor("dden", [P, QS], f32, kind="ExternalOutput")
        dua_d = nc.dram_tensor("dua", [P, QS], f32, kind="ExternalOutput")

    id_d = nc.inline_tensor(np.eye(P, dtype=np.float32), name="ident_c")
    onesb_d = nc.inline_tensor(
        np.ones((P, P), dtype=ml_dtypes.bfloat16), name="onesb_c"
    )
    onesf_d = nc.inline_tensor(np.ones((P, P), dtype=np.float32), name="onesf_c")
    ones82_np = (
        np.ones((P, 2, P), dtype=ml_dtypes.float8_e5m2)
        if av_f8 == 2
        else np.ones((P, 2, P), dtype=ml_dtypes.float8_e4m3)
    )
    ones82_d = nc.inline_tensor(ones82_np, name="ones82_c")

    x3 = x_d.ap().rearrange("(a p) n -> a p n", p=P)
    y3 = y_d.ap().rearrange("(a p) n -> a p n", p=P)
    wv3 = wv_d.ap().rearrange("(a p) c -> a p c", p=P)
    wo3 = wo_d.ap().rearrange("(a p) c -> a p c", p=P)
    bv2 = bv_d.ap().rearrange("(a p) -> a p", p=P)
    bo2 = bo_d.ap().rearrange("(a p) -> a p", p=P)

    with tile.TileContext(nc) as tc:
        with (
            tc.tile_pool(name="const", bufs=1) as constp,
            tc.tile_pool(name="xpool", bufs=1) as xpool,
            tc.tile_pool(name="wpool", bufs=1) as wpool,
            tc.tile_pool(name="big", bufs=1) as big,
            tc.tile_pool(name="expp", bufs=4) as expp,
            tc.tile_pool(name="finp", bufs=2) as finp,
            tc.tile_pool(name="dramp", bufs=2, space="DRAM") as dramp,
            tc.tile_pool(name="qkps", bufs=2, space="PSUM") as qkps,
            tc.tile_pool(name="uaps", bufs=2, space="PSUM") as uaps,
            tc.tile_pool(name="mps", bufs=1, space="PSUM") as mps,
            tc.tile_pool(name="dps", bufs=1, space="PSUM") as dps,
        ):
            # phase-0 psum scratch alternates between the two 1-bank pools
            _pctr = [0]

            def ppsum(shape):
                _pctr[0] += 1
                pool, tg = ((mps, "m"), (dps, "d"))[_pctr[0] % 2]
                return pool.tile(shape, f32, tag=tg, name=tg)

            ident = constp.tile([P, P], f32, tag="ident", name="ident")
            nc.sync.dma_start(ident[:], id_d.ap())
            ones_b = constp.tile([P, P], bf16, tag="ones_b", name="ones_b")
            nc.sync.dma_start(ones_b[:], onesb_d.ap())
            ones_f = constp.tile([P, P], f32, tag="ones_f", name="ones_f")
            nc.sync.dma_start(ones_f[:], onesf_d.ap())
            ones_82 = constp.tile(
                [P, 2, P], f8e5 if av_f8 == 2 else f8, tag="ones_82", name="ones_82"
            )
            nc.sync.dma_start(ones_82[:], ones82_d.ap())
            exb = constp.tile([P, 1], f32, tag="exb", name="exb")
            nc.vector.memset(exb[:], -4.0)
            # Trigger the exp table-set load off the critical path.
            warm = constp.tile([1, 1], f32, tag="warm", name="warm")
            nc.scalar.activation(warm[:], ident[:1, :1], AF.Exp)

            def _emit_rep():
                # ---------- phase 0: loads, weight prep, projections
                wq_sb = wpool.tile([D, C], f32, tag="wq", name="wq")
                nc.sync.dma_start(wq_sb[:], wq_d.ap())
                wk_sb = wpool.tile([D, C], f32, tag="wk", name="wk")
                nc.sync.dma_start(wk_sb[:], wk_d.ap())
                wv_sb, wo_sb = [], []
                for e in range(2):
                    t = wpool.tile([P, C], f32, tag=f"wv{e}", name=f"wv{e}")
                    nc.sync.dma_start(t[:], wv3[e])
                    wv_sb.append(t)
                    t = wpool.tile([P, C], f32, tag=f"wo{e}", name=f"wo{e}")
                    nc.sync.dma_start(t[:], wo3[e])
                    wo_sb.append(t)

                with nc.allow_non_contiguous_dma(reason="tiny bias vectors"):
                    bq_sb = wpool.tile([D, 1], f32, tag="bq", name="bq")
                    nc.sync.dma_start(bq_sb[:], bq_d.ap()[:, None])
                    bk_sb = wpool.tile([D, 1], f32, tag="bk", name="bk")
                    nc.sync.dma_start(bk_sb[:], bk_d.ap()[:, None])
                    bv_sb, bo_sb = [], []
                    for e in range(2):
                        t = wpool.tile([P, 1], f32, tag=f"bv{e}", name=f"bv{e}")
                        nc.sync.dma_start(t[:], bv2[e][:, None])
                        bv_sb.append(t)
                        t = wpool.tile([P, 1], f32, tag=f"bo{e}", name=f"bo{e}")
                        nc.sync.dma_start(t[:], bo2[e][:, None])
                        bo_sb.append(t)
                    g_sb = wpool.tile([1, 1], f32, tag="g", name="g")
                    nc.sync.dma_start(g_sb[:], g_d.ap()[:, None])

                xs, xb = [], []
                for ci in range(2):
                    t = xpool.tile([P, HW], f32, tag=f"x{ci}", name=f"x{ci}")
                    tb = xpool.tile([P, HW], bf16, tag=f"xb{ci}", name=f"xb{ci}")
                    for s in range(NSTRIP):
                        sl = slice(s * QS, (s + 1) * QS)
                        nc.gpsimd.dma_start(t[:, sl], x3[ci][:, sl])
                        nc.vector.tensor_copy(tb[:, sl], t[:, sl])
                    xs.append(t)
                    xb.append(tb)

                bqs = wpool.tile([D, 1], f32, tag="bqs", name="bqs")
                nc.scalar.mul(bqs[:], bq_sb[:], SCALE)
                # gamma clipped to [0, 1], replicated across partitions
                nc.vector.tensor_scalar(g_sb[:], g_sb[:], 1.0, 0.0, ALU.min, ALU.max)
                gd = dramp.tile([1, 1], f32, name="gd")
                nc.sync.dma_start(gd[:], g_sb[:])
                g_rep = wpool.tile([P, 1], f32, tag="grep", name="grep")
                nc.sync.dma_start(g_rep[:], gd[:].to_broadcast((P, 1)))
                gbo = []
                for e in range(2):
                    t = wpool.tile([P, 1], f32, tag=f"gbo{e}", name=f"gbo{e}")
                    nc.vector.tensor_mul(t[:], bo_sb[e][:], g_rep[:])
                    gbo.append(t)

                # transposed weights via PE transpose, cast to bf16
                wqT, wkT = [], []
                for ci in range(2):
                    ps = ppsum([P, P])
                    nc.tensor.transpose(
                        ps[:, :D], wq_sb[:, ci * P : (ci + 1) * P], ident[:D, :D]
                    )
                    t = wpool.tile([P, D], bf16, tag=f"wqT{ci}", name=f"wqT{ci}")
                    nc.vector.tensor_copy(t[:], ps[:, :D])
                    wqT.append(t)
                    ps = ppsum([P, P])
                    nc.tensor.transpose(
                        ps[:, :D], wk_sb[:, ci * P : (ci + 1) * P], ident[:D, :D]
                    )
                    t = wpool.tile([P, D], bf16, tag=f"wkT{ci}", name=f"wkT{ci}")
                    nc.vector.tensor_copy(t[:], ps[:, :D])
                    wkT.append(t)

                wvT = [
                    wpool.tile([P, C], bf16, tag=f"wvT{ci}", name=f"wvT{ci}")
                    for ci in range(2)
                ]
                woT = [
                    wpool.tile([P, C], bf16, tag=f"woT{ei}", name=f"woT{ei}")
                    for ei in range(2)
                ]
                for ci in range(2):
                    for ei in range(2):
                        ps = ppsum([P, P])
                        nc.tensor.transpose(
                            ps[:], wv_sb[ei][:, ci * P : (ci + 1) * P], ident[:]
                        )
                        nc.vector.tensor_copy(wvT[ci][:, ei * P : (ei + 1) * P], ps[:])
                        ps = ppsum([P, P])
                        nc.tensor.transpose(
                            ps[:], wo_sb[ci][:, ei * P : (ei + 1) * P], ident[:]
                        )
                        nc.vector.tensor_copy(woT[ei][:, ci * P : (ci + 1) * P], ps[:])

                # v projection (bias via ACT) -> v_sb [e, n] bf16
                v_sb = [
                    big.tile([P, HW], bf16, tag=f"v{ei}", name=f"v{ei}")
                    for ei in range(2)
                ]
                for ei in range(2):
                    for s in range(NSTRIP):
                        sl = slice(s * QS, (s + 1) * QS)
                        vps = ppsum([P, QS])
                        for ci in range(2):
                            nc.tensor.matmul(
                                vps[:],
                                wvT[ci][:, ei * P : (ei + 1) * P],
                                xb[ci][:, sl],
                                start=(ci == 0), stop=(ci == 1),
                            )
                        nc.vector.tensor_scalar_add(
                            v_sb[ei][:, sl], vps[:], bv_sb[ei][:]
                        )

                # U^T[k, f] = (Wo @ v)^T chunks. For the fp8 DoubleRow path,
                # U is split hi+lo (two fp8 terms ~ 12-bit mantissa).
                if av_f8 == 1:
                    ut_hi = big.tile([P, NKC, C], f8, tag="ut_hi", name="ut_hi")
                    ut_lo = big.tile([P, NKC, C], f8, tag="ut_lo", name="ut_lo")
                else:
                    ut = big.tile([P, NKC, C], exdt, tag="ut", name="ut")
                for kc in range(NKC):
                    ups = ppsum([P, C])
                    for ei in range(2):
                        nc.tensor.matmul(
                            ups[:],
                            v_sb[ei][:, kc * P : (kc + 1) * P],
                            woT[ei][:],
                            start=(ei == 0), stop=(ei == 1),
                        )
                    if av_f8 == 1:
                        nc.scalar.activation(ut_hi[:, kc, :], ups[:], AF.Copy)
                        nc.vector.tensor_tensor(
                            ut_lo[:, kc, :], ups[:], ut_hi[:, kc, :], ALU.subtract
                        )
                    elif av_f8 == 2:
                        nc.scalar.activation(ut[:, kc, :], ups[:], AF.Copy)
                    else:
                        nc.vector.tensor_copy(ut[:, kc, :], ups[:])

                # q/k projections -> replicated [128, HW] bf16 (4 row groups;
                # rows 16..31 of each group stay zero: contraction padded to 32)
                q_rep = big.tile([P, HW], bf16, tag="q_rep", name="q_rep")
                k_rep = big.tile([P, HW], bf16, tag="k_rep", name="k_rep")
                nc.vector.memset(q_rep[:], 0.0)
                nc.vector.memset(k_rep[:], 0.0)
                for s in range(NSTRIP):
                    sl = slice(s * QS, (s + 1) * QS)
                    qps = ppsum([D, QS])
                    for ci in range(2):
                        nc.tensor.matmul(
                            qps[:], wqT[ci][:], xb[ci][:, sl],
                            start=(ci == 0), stop=(ci == 1),
                        )
                    nc.vector.tensor_scalar(
                        q_rep[0:D, sl], qps[:], SCALE, bqs[:], ALU.mult, ALU.add
                    )
                    kps = ppsum([D, QS])
                    for ci in range(2):
                        nc.tensor.matmul(
                            kps[:], wkT[ci][:], xb[ci][:, sl],
                            start=(ci == 0), stop=(ci == 1),
                        )
                    nc.vector.tensor_scalar_add(k_rep[0:D, sl], kps[:], bk_sb[:])
                    if qkpack:
                        for r in range(1, 4):
                            nc.sync.dma_start(
                                q_rep[32 * r : 32 * r + D, sl], q_rep[0:D, sl]
                            )
                            nc.sync.dma_start(
                                k_rep[32 * r : 32 * r + D, sl], k_rep[0:D, sl]
                            )

                # ---------- phase 1: attention over q-strips
                # av_f8: PE has slack (DoubleRow AV), so QK runs unpacked from
                # row group 0 -- mixing DoubleRow with tile_position-packed
                # matmuls is avoided. bf16: 4-way row packing for PE headroom.
                def emit_qk(s, g):
                    qk = qkps.tile([P, KGRP * QS], f32, tag="qk", name="qk")
                    for j in range(KGRP):
                        kc = KGRP * g + j
                        roff = 32 * ((KGRP * g + j) % 4) if qkpack else 0
                        nc.tensor.matmul(
                            qk[:, j * QS : (j + 1) * QS],
                            k_rep[roff : roff + 32, kc * P : (kc + 1) * P],
                            q_rep[roff : roff + 32, s * QS : (s + 1) * QS],
                            start=True,
                            stop=True,
                            tile_position=(roff, 0) if qkpack else None,
                        )
                    return qk

                n_pe_den = den_pe_groups * KGRP
                use_acc = (not av_f8) and den_pe_groups < NGRP

                for s in range(NSTRIP):
                    sl = slice(s * QS, (s + 1) * QS)
                    ua = [
                        uaps.tile([P, QS], f32, tag="ua", name="ua") for _ in range(2)
                    ]
                    den = dps.tile([P, QS], f32, tag="d", name="d")
                    acc = (
                        finp.tile([P, QS], f32, tag="acc", name="acc")
                        if use_acc
                        else None
                    )
                    acc_used = False
                    den_idx = 0
                    qk = emit_qk(s, 0)
                    for g in range(NGRP):
                        ex = expp.tile([P, KGRP * QS], exdt, tag="exp", name="exp")
                        if dbg and s == 0 and g == 0:
                            dcp = finp.tile([P, KGRP * QS], f32, tag="dcp", name="dcp")
                            nc.vector.tensor_copy(dcp[:], qk[:])
                            nc.sync.dma_start(dqk_d.ap(), dcp[:])
                        if av_f8 == 1:
                            # exp(logit - 4): fits e4m3; cancels in normalize
                            nc.scalar.activation(ex[:], qk[:], AF.Exp, bias=exb[:])
                        else:
                            # e5m2 covers exp([-8.3, 9.1]) with no offset
                            nc.scalar.activation(ex[:], qk[:], AF.Exp)
                        if dbg and s == 0 and g == 0:
                            dcp2 = finp.tile([P, KGRP * QS], f32, tag="dcp", name="dcp")
                            nc.vector.tensor_copy(dcp2[:], ex[:])
                            nc.sync.dma_start(dex_d.ap(), dcp2[:])
                        if g + 1 < NGRP:
                            qk = emit_qk(s, g + 1)
                        if av_f8 == 1:
                            kc0 = KGRP * g
                            rhs2 = ex[:].rearrange("p (a q) -> p a q", a=2)
                            for fi in range(2):
                                for hl, src_t in ((0, ut_hi), (1, ut_lo)):
                                    nc.tensor.matmul(
                                        ua[fi][:],
                                        src_t[:, kc0 : kc0 + 2, fi * P : (fi + 1) * P],
                                        rhs2,
                                        start=(g == 0 and hl == 0),
                                        stop=(g == NGRP - 1 and hl == 1),
                                        perf_mode=PM.DoubleRow,
                                    )
                            nc.tensor.matmul(
                                den[:],
                                ones_82[:],
                                rhs2,
                                start=(g == 0),
                                stop=(g == NGRP - 1),
                                perf_mode=PM.DoubleRow,
                            )
                        elif av_f8 == 2:
                            kc0 = KGRP * g
                            rhs2 = ex[:].rearrange("p (a q) -> p a q", a=2)
                            for fi in range(2):
                                nc.tensor.matmul(
                                    ua[fi][:],
                                    ut[:, kc0 : kc0 + 2, fi * P : (fi + 1) * P],
                                    rhs2,
                                    start=(g == 0),
                                    stop=(g == NGRP - 1),
                                    perf_mode=PM.DoubleRow,
                                )
                            nc.tensor.matmul(
                                den[:],
                                ones_82[:],
                                rhs2,
                                start=(g == 0),
                                stop=(g == NGRP - 1),
                                perf_mode=PM.DoubleRow,
                            )
                        else:
                            for j in range(KGRP):
                                kc = KGRP * g + j
                                exj = ex[:, j * QS : (j + 1) * QS]
                                for fi in range(2):
                                    nc.tensor.matmul(
                                        ua[fi][:],
                                        ut[:, kc, fi * P : (fi + 1) * P],
                                        exj,
                                        start=(g == 0 and j == 0),
                                        stop=(g == NGRP - 1 and j == KGRP - 1),
                                    )
                                if g < den_pe_groups:
                                    nc.tensor.matmul(
                                        den[:],
                                        ones_b[:],
                                        exj,
                                        start=(den_idx == 0),
                                        stop=(not use_acc and den_idx == n_pe_den - 1),
                                    )
                                    den_idx += 1
                                else:
                                    if not acc_used:
                                        nc.vector.tensor_copy(acc[:], exj)
                                        acc_used = True
                                    else:
                                        nc.vector.tensor_add(acc[:], acc[:], exj)
                    if use_acc:
                        nc.tensor.matmul(
                            den[:],
                            ones_f[:],
                            acc[:],
                            start=(den_pe_groups == 0),
                            stop=True,
                        )

                    # epilogue: srep = 1/den (all partitions already hold den),
                    # y = (ua*srep)*gamma + gamma*bo + x
                    if dbg and s == 0:
                        dcp3 = finp.tile([P, QS], f32, tag="dcp3", name="dcp3")
                        nc.vector.tensor_copy(dcp3[:], den[:])
                        nc.sync.dma_start(dden_d.ap(), dcp3[:])
                        dcp4 = finp.tile([P, QS], f32, tag="dcp3", name="dcp3")
                        nc.vector.tensor_copy(dcp4[:], ua[0][:])
                        nc.sync.dma_start(dua_d.ap(), dcp4[:])
                    srep = finp.tile([P, QS], f32, tag="srep", name="srep")
                    nc.vector.reciprocal(srep[:], den[:])
                    for fi in range(2):
                        yt = finp.tile([P, QS], f32, tag="yt", name="yt")
                        nc.vector.tensor_mul(yt[:], ua[fi][:], srep[:])
                        nc.vector.tensor_scalar(
                            yt[:], yt[:], g_rep[:], gbo[fi][:], ALU.mult, ALU.add
                        )
                        nc.vector.tensor_add(yt[:], yt[:], xs[fi][:, sl])
                        nc.sync.dma_start(y3[fi, :, sl], yt[:])

            if loop_repeat:
                with tc.For_i(0, loop_repeat):
                    _emit_rep()
            else:
                for _ in range(repeat):
                    _emit_rep()

    nc.compile()
    return nc


def _get_nc(**kw):
    key = tuple(sorted(kw.items()))
    if key not in _cache:
        _cache[key] = _build(**kw)
    return _cache[key]


def _in_maps(inputs):
    names = ["Wq", "bq", "Wk", "bk", "Wv", "bv", "Wo", "bo", "gamma"]
    base = {
        n: np.ascontiguousarray(np.asarray(inputs[n], dtype=np.float32))
        for n in names
    }
    x = np.ascontiguousarray(np.asarray(inputs["x"], dtype=np.float32))
    assert x.shape == (B, C, 64, 64), x.shape
    maps = []
    for b in range(B):
        m = dict(base)
        m["x"] = np.ascontiguousarray(x[b].reshape(C, HW))
        maps.append(m)
    return maps


def _run(inputs, trace=False, build_kw=None, **kw):
    from concourse.bass_utils import run_bass_kernel_spmd

    nc = _get_nc(**(build_kw or {}))
    res = run_bass_kernel_spmd(
        nc, _in_maps(inputs), core_ids=list(range(B)), trace=trace, **kw
    )
    y = np.stack([r["y"] for r in res.results]).reshape(B, C, 64, 64)
    return np.ascontiguousarray(y.astype(np.float32)), res


def kernel(**inputs):
    y, _ = _run(inputs)
    return y

